# revision 1
# baseline (speedup 1.0000x reference)
"""NT-Xent contrastive loss (SimCLR-style) on 8 Trainium2 NeuronCores.

Problem: z1, z2 [4096, 256] fp32 -> scalar loss.
  zn = l2norm(z), z = concat(z1, z2) -> [8192, 256]
  sim = zn @ zn.T / 0.07              -> [8192, 8192]
  loss = -mean_i log_softmax(sim)[i, partner(i)],  partner(i) = (i + 4096) % 8192

Sharding: data-parallel over rows. Each core owns 1024 query rows and computes
their 1024 x 8192 similarity block against all keys, exp + row-sum fused on the
scalar engine, and the target-pair diagonal. Host sums the 8 per-core partials.

Per-core input layout trick: each core receives the (normalized, transposed)
embedding matrix ROLLED so its own queries sit at columns [0, 1024) and their
partners at [4096, 5120). The softmax denominator is permutation-invariant, so
rolling is free and lets all 8 cores run one identical SPMD program.
"""

import numpy as np

import concourse.bass as bass
import concourse.tile as tile
from concourse import bacc, mybir
from concourse.bass_utils import run_bass_kernel_spmd

B, D = 4096, 256
N = 2 * B            # 8192 embeddings
NCORES = 8
QPC = N // NCORES    # 1024 query rows per core
TEMP = 0.07

F32 = mybir.dt.float32
F32R = mybir.dt.float32r

KG = 4               # key groups per core
KGW = N // KG        # 2048 keys per group (4 PSUM banks)
QT = QPC // 128      # 8 query tiles of 128 rows


def build_nc() -> bass.Bass:
    nc = bacc.Bacc("TRN2", target_bir_lowering=False, debug=False, num_devices=NCORES)
    ztn = nc.declare_dram_parameter("ztn", [D, N], F32R, isOutput=False)
    ident = nc.declare_dram_parameter("ident", [128, 128], F32, isOutput=False)
    part = nc.declare_dram_parameter("part", [128, QT], F32, isOutput=True)

    with tile.TileContext(nc) as tc:
        with (
            tc.tile_pool(name="zt", bufs=1) as zt_pool,
            tc.tile_pool(name="const", bufs=1) as const_pool,
            tc.tile_pool(name="gram", bufs=2, space="PSUM") as gram_pool,
            tc.tile_pool(name="expout", bufs=3) as exp_pool,
            tc.tile_pool(name="diag", bufs=2) as diag_pool,
            tc.tile_pool(name="stats", bufs=1) as stats_pool,
        ):
            # Normalized z^T per (key-group, d-chunk): 8 tiles [128, 2048]
            # Separate tiles keep each matmul's dep list to its own DMA.
            idt = const_pool.tile([128, 128], F32)
            nc.sync.dma_start(idt[:], ident[:])
            ztg = [[None, None] for _ in range(KG)]
            dma_insts = {}   # (g, h) -> [inst for c in 0,1], h = which half
            half = KGW // 2
            for g in range(KG):
                for c in range(2):
                    t = zt_pool.tile([128, KGW], F32R, tag=f"zt{g}_{c}")
                    ztg[g][c] = t
                # issue both d-chunks of half h before half h+1 so the first
                # matmuls (which need both d-chunks) start sooner
                for h in range(2):
                    ds = []
                    for c in range(2):
                        d = nc.sync.dma_start(
                            ztg[g][c][:, h * half:(h + 1) * half],
                            ztn[
                                c * 128:(c + 1) * 128,
                                g * KGW + h * half: g * KGW + (h + 1) * half,
                            ],
                        )
                        ds.append(d.ins)
                    dma_insts[(g, h)] = ds

            # accumulators: exp row-sums per (q-tile, key-group); target sims
            sums = stats_pool.tile([128, QT, KG], F32)
            tgt = stats_pool.tile([128, QT], F32)
            S = stats_pool.tile([128, QT], F32)
            logS = stats_pool.tile([128, QT], F32)
            out_t = stats_pool.tile([128, QT], F32)

            # The S3_LW lowering of a self-loading matmul fits only ONE sync
            # wait, so route every multi-wait dependency through a PE nop
            # "gate" that precedes each PSUM tile's first matmul.
            iter_readers = []  # per (g,q): insts whose psum slot gets reused
            idx = 0
            for g in range(KG):
                for q in range(QT):
                    ps = gram_pool.tile([128, KGW], F32)
                    # gram block: queries (stationary) x 2048 keys, contract D=256
                    last_mm = None
                    for kc in range(KGW // 512):
                        for c in range(2):
                            mm = nc.tensor.matmul(
                                ps[:, kc * 512:(kc + 1) * 512],
                                lhsT=ztg[0][c][:, q * 128:(q + 1) * 128],
                                rhs=ztg[g][c][:, kc * 512:(kc + 1) * 512],
                                start=(c == 0),
                                stop=(c == 1),
                            )
                            last_mm = mm
                    readers = [last_mm.ins]
                    # target pairs live at key cols 4096 + 128q (key group 2)
                    if g == 2:
                        dg = diag_pool.tile([128, 128], F32)
                        di = nc.vector.tensor_mul(
                            dg[:], ps[:, q * 128:(q + 1) * 128], idt[:],
                        )
                        nc.vector.tensor_reduce(
                            out=tgt[:, q:q + 1], in_=dg[:],
                            axis=mybir.AxisListType.X, op=mybir.AluOpType.add,
                        )
                        readers.append(di.ins)
                    # exp(sim/temp) in place in PSUM, fused row-sum accumulation
                    ei = nc.scalar.activation(
                        ps[:],
                        ps[:],
                        mybir.ActivationFunctionType.Exp,
                        scale=1.0 / TEMP,
                        accum_out=sums[:, q, g:g + 1],
                    )
                    readers.append(ei.ins)
                    iter_readers.append(readers)
                    idx += 1

            # logsumexp_i - sim_target_i, summed later on host
            nc.vector.tensor_reduce(
                out=S[:], in_=sums[:], axis=mybir.AxisListType.X,
                op=mybir.AluOpType.add,
            )
            nc.scalar.activation(logS[:], S[:], mybir.ActivationFunctionType.Ln)
            # tgt holds raw target-pair dots; scale by 1/TEMP before subtract
            tgt_s = stats_pool.tile([128, QT], F32)
            nc.scalar.mul(tgt_s[:], tgt[:], 1.0 / TEMP)
            nc.vector.tensor_sub(out_t[:], logS[:], tgt_s[:])
            nc.sync.dma_start(part[:], out_t[:])

    nc.compile()
    return nc


_NC_CACHE: list = []


def kernel(z1: np.ndarray, z2: np.ndarray) -> np.ndarray:
    z = np.concatenate([np.asarray(z1), np.asarray(z2)], axis=0).astype(np.float32)
    zn = z / np.maximum(np.linalg.norm(z, axis=1, keepdims=True), 1e-12)
    ztn = np.ascontiguousarray(zn.T)  # [256, 8192]
    ident = np.eye(128, dtype=np.float32)

    in_maps = [
        {
            "ztn": np.ascontiguousarray(np.roll(ztn, -QPC * c, axis=1)),
            "ident": ident,
        }
        for c in range(NCORES)
    ]

    if not _NC_CACHE:
        _NC_CACHE.append(build_nc())
    nc = _NC_CACHE[0]

    res = run_bass_kernel_spmd(nc, in_maps, list(range(NCORES)))
    total = 0.0
    for c in range(NCORES):
        total += res.results[c]["part"].astype(np.float64).sum()
    return np.float32(total / N)


if __name__ == "__main__":
    rng = np.random.default_rng(0)
    z1 = rng.standard_normal((B, D), dtype=np.float32)
    z2 = rng.standard_normal((B, D), dtype=np.float32)
    print(kernel(z1, z2))



# revision 33
# speedup vs baseline: 1.0250x; 1.0250x over previous
"""NT-Xent contrastive loss (SimCLR-style) on 8 Trainium2 NeuronCores.

Problem: z1, z2 [4096, 256] fp32 -> scalar loss.
  zn = l2norm(z), z = concat(z1, z2) -> [8192, 256]
  sim = zn @ zn.T / 0.07              -> [8192, 8192]
  loss = -mean_i log_softmax(sim)[i, partner(i)],  partner(i) = (i + 4096) % 8192

Strategy (symmetric): exp(sim) is symmetric, so each unordered tile pair
{a, b} of the 64x64 grid of 128x128 blocks is computed ONCE. The core
owning row-tile a computes blocks (a, a+o mod 64) for o = 0..32 (the
o=32 pair is computed by both owners: 3% redundancy that keeps the
program SPMD-identical). Row sums of each exp block accumulate directly
(DVE reduce); the transpose credit for o = 1..31 comes from COLUMN sums
of the same exp block, computed on the otherwise-idle PE as ones^T @ E
matmuls accumulating in PSUM. Host adds the per-core partial sums.

Per-core input is the normalized z^T rolled so its own 1024 rows sit at
columns [0, 1024): every core runs one identical program, and the o-arcs
become contiguous column ranges [0, 5120) -- only 62.5% of z is even
loaded. Matmuls run in fp8e4m3 (values pre-scaled x16) with DoubleRow
packing K=256 into one pass; exp runs on the scalar engine PSUM->SBUF in
bf16. Tolerance is rel 2e-2; fp8 error lands ~1e-3.

PSUM discipline: an accumulation group conflicts with any other group
opened in the same bank while it is live (start zero-marks the whole
2KB bank row), so long-lived accumulators get exclusive banks. Banks
0-3: gram double-buffer ([128,1024] x 2). Banks 4-7: col-sum chunks
cc=2..5 (target tiles 8..23), held open across the whole q loop.
Chunks cc=6,7 and the 7 inter-core edge tiles (t=32..38) run at the
tail through the drained banks; the 7 intra-core edge tiles (t=1..7)
run mid-loop as brief transient groups inside a gram slot.

exp SBUF layout per q: [o32 | o0 | o1 | ... | o31], so the o=32 block
rides in gram block 0 and every ACTIVATE is a full [128, 1024].
"""

import numpy as np

import concourse.bass as bass
import concourse.tile as tile
from concourse import bacc, mybir
from concourse.bass_utils import run_bass_kernel_spmd

B, D = 4096, 256
N = 2 * B            # 8192 embeddings
NCORES = 8
NT = N // 128        # 64 tiles of 128 embeddings
Q = 8                # row tiles per core
ARC = 33             # column tiles per row tile (o = 0..32)
COLS = ARC * 128     # 4224
ZCOLS = (Q - 1 + 32 + 1) * 128   # 5120 rolled columns needed per core
TEMP = 0.07
ZSCALE = 16.0        # pre-scale before fp8 cast (keeps values in e4m3 normal range)
EXP_SCALE = 1.0 / (ZSCALE * ZSCALE * TEMP)

USE_FP8 = True       # fp8e4m3 + DoubleRow; False -> bf16 two-pass contraction

F32 = mybir.dt.float32
BF16 = mybir.dt.bfloat16
FP8 = mybir.dt.float8e4

EDGE_TILES = list(range(1, 8)) + list(range(32, 39))


PROBE = "full"   # hw_probe.py bisect knob: full | nocolsum | noreduce | min


def build_nc() -> bass.Bass:
    probe = PROBE
    do_colsum = probe in ("full", "noreduce", "nottr", "norowred")
    do_ttr = probe in ("full", "nocolsum", "norowred")
    do_rowred = probe in ("full", "nocolsum", "nottr")
    do_dve = do_ttr or do_rowred
    zdt = FP8 if USE_FP8 else BF16
    nc = bacc.Bacc("TRN2", target_bir_lowering=False, debug=False, num_devices=NCORES)
    zdr = nc.declare_dram_parameter("zdr", [128, 2, ZCOLS], zdt, isOutput=False)
    idt_d = nc.declare_dram_parameter("idt", [128, 128], F32, isOutput=False)
    ones_d = nc.declare_dram_parameter("ones", [128, 32], BF16, isOutput=False)
    rowsum_d = nc.declare_dram_parameter("rowsum", [128, Q], F32, isOutput=True)
    tgt_d = nc.declare_dram_parameter("tgt", [128, Q], F32, isOutput=True)
    mid_d = nc.declare_dram_parameter("mid", [1, 6 * 512], F32, isOutput=True)
    edge_d = nc.declare_dram_parameter("edge", [1, 14 * 128], F32, isOutput=True)

    with tile.TileContext(nc) as tc:
        with (
            tc.tile_pool(name="zp", bufs=1) as zp,
            tc.tile_pool(name="const", bufs=1) as constp,
            tc.tile_pool(name="expp", bufs=8) as expp,
            tc.tile_pool(name="dgp", bufs=2) as dgp,
            tc.tile_pool(name="stats", bufs=1) as statsp,
            tc.tile_pool(name="gram", bufs=2, space="PSUM") as gramp,
            tc.tile_pool(name="cacc", bufs=1, space="PSUM") as caccp,
        ):
            z = zp.tile([128, 2, ZCOLS], zdt)
            for i in range(4):
                w = ZCOLS // 4
                nc.sync.dma_start(
                    z[:, :, i * w:(i + 1) * w], zdr[:, :, i * w:(i + 1) * w]
                )
            idt = constp.tile([128, 128], F32)
            nc.sync.dma_start(idt[:], idt_d[:])
            ones = constp.tile([128, 32], BF16)
            nc.sync.dma_start(ones[:], ones_d[:])

            rowsum = statsp.tile([128, Q], F32)
            tgtt = statsp.tile([128, Q], F32)
            edgestage = statsp.tile([32, 14 * 128], F32)
            midstage = statsp.tile([32, 6 * 512], F32)
            zeros = statsp.tile([32, 512], F32)
            nc.any.memset(zeros[:], 0.0)
            # long-lived col-sum accumulator banks (chunks cc=2..5; banks
            # reused at the tail for cc=6,7 and the t>=32 edge tiles)
            macc = [
                caccp.tile([128, 512], F32, tag=f"M{i}", name=f"macc{i}")
                for i in range(4)
            ]

            def gram_mm(out_ap, qs, c0, c1):
                """out = z[:, qs:qs+128].T @ z[:, c0:c1] (scaled x256)."""
                if USE_FP8:
                    nc.tensor.matmul(
                        out_ap,
                        lhsT=z[:, :, qs:qs + 128],
                        rhs=z[:, :, c0:c1],
                        start=True,
                        stop=True,
                        perf_mode=mybir.MatmulPerfMode.DoubleRow,
                    )
                else:
                    for ko in range(2):
                        nc.tensor.matmul(
                            out_ap,
                            lhsT=z[:, ko, qs:qs + 128],
                            rhs=z[:, ko, c0:c1],
                            start=(ko == 0),
                            stop=(ko == 1),
                        )

            def dve_copy(dst, src):
                # PSUM -> SBUF drain; only one non-scalar input may be PSUM
                w = src.shape[-1]
                nc.vector.scalar_tensor_tensor(
                    out=dst,
                    in0=src,
                    scalar=0.0,
                    in1=zeros[:, 0:w],
                    op0=mybir.AluOpType.bypass,
                    op1=mybir.AluOpType.add,
                )

            ets = []

            def emit_gram(q):
                qs = 128 * q
                et = expp.tile([128, COLS], BF16, tag="et")
                ets.append(et)
                # block 0: [o32 | o0..o6.75] -> et[:, 0:1024]
                ps = gramp.tile([128, 1024], F32, tag="g")
                gram_mm(ps[:, 0:128], qs, qs + 4096, qs + 4224)      # o=32
                gram_mm(ps[:, 128:512], qs, qs, qs + 384)
                gram_mm(ps[:, 512:1024], qs, qs + 384, qs + 896)
                if do_ttr:
                    # target-pair diag of the o=32 block: mask with identity,
                    # then row-reduce (TensorTensorReduce crashes the device
                    # in this config, so use the two-instruction form)
                    dg = dgp.tile([128, 128], F32, tag="dg")
                    nc.vector.tensor_mul(dg[:], ps[:, 0:128], idt[:])
                    nc.vector.tensor_reduce(
                        out=tgtt[:, q:q + 1],
                        in_=dg[:],
                        axis=mybir.AxisListType.X,
                        op=mybir.AluOpType.add,
                    )
                nc.scalar.activation(
                    et[:, 0:1024], ps[:],
                    mybir.ActivationFunctionType.Exp, scale=EXP_SCALE,
                )
                for b in range(1, 4):
                    ps = gramp.tile([128, 1024], F32, tag="g")
                    base = qs + 896 + 1024 * (b - 1)
                    gram_mm(ps[:, 0:512], qs, base, base + 512)
                    gram_mm(ps[:, 512:1024], qs, base + 512, base + 1024)
                    nc.scalar.activation(
                        et[:, 1024 * b:1024 * (b + 1)], ps[:],
                        mybir.ActivationFunctionType.Exp, scale=EXP_SCALE,
                    )
                # arc tail x in [3968, 4096) (tile o=31)
                ps = gramp.tile([128, 1024], F32, tag="g")
                gram_mm(ps[:, 0:128], qs, qs + 3968, qs + 4096)
                nc.scalar.activation(
                    et[:, 4096:4224], ps[:, 0:128],
                    mybir.ActivationFunctionType.Exp, scale=EXP_SCALE,
                )
                if do_rowred:
                    nc.vector.tensor_reduce(
                        out=rowsum[:, q:q + 1],
                        in_=et[:],
                        axis=mybir.AxisListType.X,
                        op=mybir.AluOpType.add,
                    )

            # et offset of arc column x (x = rolled col - qs), o>=0 tiles
            # shifted +128 by the leading o32 block: et_off = x + 128
            def colsum(granule, tq, x0, w, start, stop):
                nc.tensor.matmul(
                    granule,
                    lhsT=ones[:],
                    rhs=ets[tq][:, x0 + 128:x0 + 128 + w],
                    start=start,
                    stop=stop,
                )

            def emit_mid_colsums(q):
                """Chunks cc=2..5 (target tiles 8..23): every q contributes;
                PE accumulates across q in 4 exclusive PSUM banks."""
                for cc in range(2, 6):
                    colsum(
                        macc[cc - 2][0:32, 0:512], q, 512 * cc - 128 * q, 512,
                        start=(q == 0), stop=(q == 7),
                    )

            def emit_edge(t, e, granule):
                """All contributions for edge tile t, back-to-back (brief
                transient group), then drained to SBUF staging."""
                lo = max(0, t - 31)
                hi = min(7, t - 1)
                for q2 in range(lo, hi + 1):
                    colsum(
                        granule, q2, 128 * (t - q2), 128,
                        start=(q2 == lo), stop=(q2 == hi),
                    )
                dve_copy(edgestage[:, 128 * e:128 * (e + 1)], granule)

            for q in range(Q):
                emit_gram(q)
                if q >= 1 and do_colsum:
                    emit_mid_colsums(q - 1)
                    # intra-core edge tile t=q needs ets[0..q-1] only
                    g = gramp.tile([128, 1024], F32, tag="g")
                    emit_edge(q, q - 1, g[0:32, 0:128])
            if do_colsum:
                emit_mid_colsums(Q - 1)

                # tail: drain cc=2..5, then run cc=6,7 + edges t=32..38
                # through the freed banks
                for cc in range(2, 6):
                    dve_copy(
                        midstage[:, 512 * (cc - 2):512 * (cc - 1)],
                        macc[cc - 2][0:32, 0:512],
                    )
                for cc in (6, 7):
                    gran = macc[cc - 6][0:32, 0:512]
                    for q2 in range(Q):
                        colsum(
                            gran, q2, 512 * cc - 128 * q2, 512,
                            start=(q2 == 0), stop=(q2 == 7),
                        )
                    dve_copy(midstage[:, 512 * (cc - 2):512 * (cc - 1)], gran)
                for e, t in enumerate(EDGE_TILES):
                    if t < 32:
                        continue   # done mid-loop
                    gran = macc[2 + (e % 2)][0:32, 0:128]
                    emit_edge(t, e, gran)

                nc.sync.dma_start(edge_d[:], edgestage[0:1, :])
                nc.sync.dma_start(mid_d[:], midstage[0:1, :])
            if not do_rowred:
                # probe mode: keep outputs written so the build passes
                nc.scalar.copy(rowsum[:], ets[0][:, 0:Q])
            if not do_ttr:
                nc.scalar.copy(tgtt[:], ets[0][:, 0:Q])
            nc.sync.dma_start(rowsum_d[:], rowsum[:])
            nc.sync.dma_start(tgt_d[:], tgtt[:])

    nc.compile()
    return nc


def make_in_maps(z1: np.ndarray, z2: np.ndarray) -> list[dict]:
    z = np.concatenate([np.asarray(z1), np.asarray(z2)], axis=0).astype(np.float64)
    zn = z / np.maximum(np.linalg.norm(z, axis=1, keepdims=True), 1e-12)
    ztn = np.ascontiguousarray(zn.T * ZSCALE)  # [256, 8192]
    zdt = mybir.dt.np(FP8 if USE_FP8 else BF16)
    ident = np.eye(128, dtype=np.float32)
    onesm = np.ones((128, 32), dtype=mybir.dt.np(BF16))
    in_maps = []
    for c in range(NCORES):
        rolled = np.roll(ztn, -1024 * c, axis=1)[:, :ZCOLS]
        # DoubleRow layout: [partition p, ko, x] = row (128*ko + p)
        zdr = np.ascontiguousarray(
            rolled.reshape(2, 128, ZCOLS).transpose(1, 0, 2)
        ).astype(zdt)
        in_maps.append({"zdr": zdr, "idt": ident, "ones": onesm})
    return in_maps


def assemble(results: list[dict]) -> np.float32:
    S = np.zeros(N, dtype=np.float64)
    tgt_all = np.zeros(N, dtype=np.float64)
    for c in range(NCORES):
        r = results[c]
        rowsum = r["rowsum"].astype(np.float64)
        tgt = r["tgt"].astype(np.float64)
        mid = r["mid"].astype(np.float64).reshape(6, 512)
        edge = r["edge"].astype(np.float64).reshape(14, 128)
        base = 1024 * c
        for q in range(Q):
            S[base + 128 * q: base + 128 * (q + 1)] += rowsum[:, q]
            tgt_all[base + 128 * q: base + 128 * (q + 1)] = tgt[:, q]
        for cc in range(2, 8):
            gidx = (512 * cc + np.arange(512) + base) % N
            S[gidx] += mid[cc - 2]
        for e, t in enumerate(EDGE_TILES):
            gidx = (128 * t + np.arange(128) + base) % N
            S[gidx] += edge[e]
    loss = np.mean(np.log(S) - tgt_all / (ZSCALE * ZSCALE * TEMP))
    return np.float32(loss)


_NC_CACHE: list = []


def kernel(z1: np.ndarray, z2: np.ndarray) -> np.ndarray:
    in_maps = make_in_maps(z1, z2)
    if not _NC_CACHE:
        _NC_CACHE.append(build_nc())
    nc = _NC_CACHE[0]
    res = run_bass_kernel_spmd(nc, in_maps, list(range(NCORES)))
    return assemble(res.results)


if __name__ == "__main__":
    rng = np.random.default_rng(0)
    z1 = rng.standard_normal((B, D), dtype=np.float32)
    z2 = rng.standard_normal((B, D), dtype=np.float32)
    print(kernel(z1, z2))
